# revision 21
# baseline (speedup 1.0000x reference)
"""ComplexPolarAttention Trainium2 kernel (8-core SPMD, row-sharded), v3.

Math (matching the reference):
  c = mag*cos(phase); s = mag*sin(phase)
  scores = c@c.T + s@s.T + bias     (bias: sparse edge scatter, last-dup-wins)
  attn = softmax(scores, axis=1)
  out = (attn@mag, attn@phase)

v3 design:
  - Device computes UNBIASED attention sums only: P = bf16(exp(S)),
    out = [P.T@mag | P.T@phase | P.T@1] unnormalized (129 cols incl Z).
  - The sparse edge bias is applied ON HOST as an exact correction: for each
    (deduped) edge cell, subtract the device's emulated unbiased weight
    bf16(exp(S16)) and add the true biased weight exp(S16 + es), both against
    the same value vectors. The ACT Exp table matches np.exp to ~1e-5 and the
    f16/bf16 roundings are emulated bit-exactly, so the residual is tiny.
  - Keys use a permuted order: node n -> (tile j = n%64, lane p = n//64) so
    mag/phase load as one contiguous 16KB descriptor per partition.
  - QK software-pipelined 2 groups ahead of PV on the PE; exp runs directly
    PSUM->SBUF(bf16) on the scalar engine (the throughput limiter).
"""
import os
import sys

sys.path.insert(0, "/opt/trn_rl_repo")

# The NTFF profile hook module is missing from this image's antenv package;
# bass_utils imports it unconditionally when tracing. Create it if absent so
# BASS_TRACE=1 works (degrades silently if dirs are read-only).
_HOOK_SRC = '''_hook = None

def set_axon_ntff_profile_hook(hook):
    global _hook
    _hook = hook

def get_axon_ntff_profile_hook():
    return _hook
'''
for _d in ("/opt/trn_rl_repo/antenv", "/root/.axon_site/_ro/trn_rl_repo/antenv"):
    try:
        _p = os.path.join(_d, "axon_hooks.py")
        if os.path.isdir(_d) and not os.path.exists(_p):
            with open(_p, "w") as _f:
                _f.write(_HOOK_SRC)
    except OSError:
        pass

import math
import numpy as np
import ml_dtypes

import concourse.bass as bass
import concourse.mybir as mybir
import concourse.tile as tile
from concourse import bacc
from concourse.bass_utils import run_bass_kernel_spmd
from concourse.masks import make_identity

N, D, E, EDGE_DIM = 8192, 64, 262144, 4
CORES = 8
NQ = N // CORES          # 1024 query rows per core
QB_W = 256               # query block width
N_QB = NQ // QB_W        # 4 query blocks per core
NT = 64                  # key tiles (128 permuted nodes each)
KCG = 4                  # key tiles per group
N_G = NT // KCG          # 16 groups
GW = KCG * QB_W          # 1024 = group tile width
MPW = 132                # padded [mag|phase|ones] chunk stride
NCHUNK = 4               # key trig chunks (16 tiles each)
CJT = NT // NCHUNK       # tiles per chunk

f32 = mybir.dt.float32
f16 = mybir.dt.float16
bf16 = mybir.dt.bfloat16
AF = mybir.ActivationFunctionType
ALU = mybir.AluOpType

_cache = {}
LAST_RESULTS = None


def _build():
    nc = bacc.Bacc("TRN2", target_bir_lowering=False, debug=False,
                   num_devices=CORES)
    mag_d = nc.dram_tensor("mag", (N, D), f32, kind="ExternalInput")
    phase_d = nc.dram_tensor("phase", (N, D), f32, kind="ExternalInput")
    magq_d = nc.dram_tensor("magq", (NQ, D), f32, kind="ExternalInput")
    phaseq_d = nc.dram_tensor("phaseq", (NQ, D), f32, kind="ExternalInput")
    out_d = nc.dram_tensor("out", (NQ, 129), f32, kind="ExternalOutput")

    with tile.TileContext(nc) as tc, \
         tc.tile_pool(name="persist", bufs=1) as pers:
        xt = pers.tile([128, N], f16, tag="xt")          # keys, feature-major
        xtq = pers.tile([128, NQ], f16, tag="xtq")       # queries, feature-major
        mp = pers.tile([128, NT * MPW], bf16, tag="mp")  # [mag|phase|1] per tile
        ident = pers.tile([128, 128], f16, tag="ident")
        make_identity(nc, ident[:])

        # ---- stage A ------------------------------------------------------
        with tc.tile_pool(name="a_big", bufs=1) as bigp, \
             tc.tile_pool(name="a_q", bufs=1) as qp, \
             tc.tile_pool(name="a_ws", bufs=2) as wsp, \
             tc.tile_pool(name="a_trig", bufs=2) as trp, \
             tc.tile_pool(name="a_cs", bufs=2) as csp, \
             tc.tile_pool(name="a_ps", bufs=3, space="PSUM") as apsp:
            # queries: [p = i%128, t = i//128, d] (256B descriptors, small)
            magqb = qp.tile([128, 8 * D], f32, tag="magqb")
            phqb = qp.tile([128, 8 * D], f32, tag="phqb")
            magq_r = magq_d[:].rearrange("(t p) d -> p t d", p=128)
            phq_r = phaseq_d[:].rearrange("(t p) d -> p t d", p=128)
            nc.sync.dma_start(out=magqb[:].rearrange("p (t d) -> p t d", d=D),
                              in_=magq_r)
            nc.sync.dma_start(out=phqb[:].rearrange("p (t d) -> p t d", d=D),
                              in_=phq_r)

            # keys: contiguous per-partition load, node n at (p=n//64, j=n%64)
            magb = bigp.tile([128, NT * D], f32, tag="magb")
            phb = bigp.tile([128, NT * D], f32, tag="phb")
            mag_r = mag_d[:].rearrange("(p j) d -> p j d", p=128)
            pha_r = phase_d[:].rearrange("(p j) d -> p j d", p=128)
            magb3 = magb[:].rearrange("p (j d) -> p j d", d=D)
            phb3 = phb[:].rearrange("p (j d) -> p j d", d=D)
            for h in range(NCHUNK):
                j0, j1 = h * CJT, (h + 1) * CJT
                nc.gpsimd.dma_start(out=magb3[:, j0:j1, :],
                                    in_=mag_r[:, j0:j1, :])
                nc.gpsimd.dma_start(out=phb3[:, j0:j1, :],
                                    in_=pha_r[:, j0:j1, :])

            # query trig -> csq -> 8 transposes -> xtq
            magqb3 = magqb[:].rearrange("p (t d) -> p t d", d=D)
            phqb3 = phqb[:].rearrange("p (t d) -> p t d", d=D)
            maghq = qp.tile([128, 8 * D], f16, tag="maghq")
            nc.gpsimd.tensor_copy(out=maghq[:], in_=magqb[:])
            maghq3 = maghq[:].rearrange("p (t d) -> p t d", d=D)
            wsq = qp.tile([128, 8 * 128], f32, tag="wsq")
            wsq3 = wsq[:].rearrange("p (t x) -> p t x", x=128)
            nc.vector.add_range_wrap(out=wsq3[:, :, 0:D], in_=phqb3[:],
                                     shift=math.pi / 2, bound=math.pi,
                                     period=2 * math.pi)
            nc.vector.add_range_wrap(out=wsq3[:, :, D:2 * D], in_=phqb3[:],
                                     shift=0.0, bound=math.pi,
                                     period=2 * math.pi)
            trigq = qp.tile([128, 8 * 128], f16, tag="trigq")
            nc.scalar.activation(out=trigq[:], in_=wsq[:], func=AF.Sin)
            trigq3 = trigq[:].rearrange("p (t x) -> p t x", x=128)
            csq = qp.tile([128, 8 * 128], f16, tag="csq")
            csq3 = csq[:].rearrange("p (t x) -> p t x", x=128)
            nc.vector.tensor_tensor(out=csq3[:, :, 0:D], in0=trigq3[:, :, 0:D],
                                    in1=maghq3[:], op=ALU.mult)
            nc.vector.tensor_tensor(out=csq3[:, :, D:2 * D],
                                    in0=trigq3[:, :, D:2 * D],
                                    in1=maghq3[:], op=ALU.mult)
            for b in range(2):
                tp = apsp.tile([128, 512], f16, tag="tpq")
                for t in range(4):
                    i = b * 4 + t
                    nc.tensor.transpose(out=tp[:, t * 128:(t + 1) * 128],
                                        in_=csq[:, i * 128:(i + 1) * 128],
                                        identity=ident[:])
                nc.vector.tensor_copy(out=xtq[:, b * 512:(b + 1) * 512],
                                      in_=tp[:])

            # key trig in chunks; ARW (f32) -> Sin (f32->f16) -> f16 mults;
            # transposes in batches of 4, PSUM->SBUF copies on gpsimd; mp
            # slices cast on scalar per chunk.
            mp3 = mp[:].rearrange("p (j c) -> p j c", c=MPW)
            for h in range(NCHUNK):
                j0 = h * CJT
                magh = csp.tile([128, CJT * D], f16, tag="magh")
                nc.gpsimd.tensor_copy(out=magh[:], in_=magb[:, j0 * D:(j0 + CJT) * D])
                magh3 = magh[:].rearrange("p (j d) -> p j d", d=D)
                ws = wsp.tile([128, CJT * 128], f32, tag="ws")
                ws3 = ws[:].rearrange("p (j x) -> p j x", x=128)
                nc.vector.add_range_wrap(out=ws3[:, :, 0:D],
                                         in_=phb3[:, j0:j0 + CJT, :],
                                         shift=math.pi / 2, bound=math.pi,
                                         period=2 * math.pi)
                nc.vector.add_range_wrap(out=ws3[:, :, D:2 * D],
                                         in_=phb3[:, j0:j0 + CJT, :],
                                         shift=0.0, bound=math.pi,
                                         period=2 * math.pi)
                trig = trp.tile([128, CJT * 128], f16, tag="trig")
                nc.scalar.activation(out=trig[:], in_=ws[:], func=AF.Sin)
                trig3 = trig[:].rearrange("p (j x) -> p j x", x=128)
                cs = csp.tile([128, CJT * 128], f16, tag="cs")
                cs3 = cs[:].rearrange("p (j x) -> p j x", x=128)
                nc.vector.tensor_tensor(out=cs3[:, :, 0:D],
                                        in0=trig3[:, :, 0:D],
                                        in1=magh3[:], op=ALU.mult)
                nc.vector.tensor_tensor(out=cs3[:, :, D:2 * D],
                                        in0=trig3[:, :, D:2 * D],
                                        in1=magh3[:], op=ALU.mult)
                for b in range(CJT // 4):
                    tp = apsp.tile([128, 512], f16, tag="tpk")
                    for t in range(4):
                        i = b * 4 + t
                        nc.tensor.transpose(out=tp[:, t * 128:(t + 1) * 128],
                                            in_=cs[:, i * 128:(i + 1) * 128],
                                            identity=ident[:])
                    dst = xt[:, (j0 + b * 4) * 128:(j0 + b * 4 + 4) * 128]
                    if b % 2 == 0:
                        nc.scalar.copy(out=dst, in_=tp[:])
                    else:
                        nc.vector.tensor_copy(out=dst, in_=tp[:])
                # mp slices for this chunk ([mag|phase|1] per key tile)
                nc.gpsimd.tensor_copy(out=mp3[:, j0:j0 + CJT, 0:D],
                                      in_=magb3[:, j0:j0 + CJT, :])
                nc.gpsimd.tensor_copy(out=mp3[:, j0:j0 + CJT, D:2 * D],
                                      in_=phb3[:, j0:j0 + CJT, :])
                nc.vector.memset(mp3[:, j0:j0 + CJT, 2 * D:2 * D + 1], 1.0)

        # ---- stage B: QK -> exp -> PV (QK pipelined 2 groups ahead)
        with tc.tile_pool(name="b_psb", bufs=2) as psbp, \
             tc.tile_pool(name="b_qk", bufs=3, space="PSUM") as qkpp, \
             tc.tile_pool(name="b_pv", bufs=1, space="PSUM") as pvpp, \
             tc.tile_pool(name="b_ep", bufs=2) as ep2:
            for qb in range(N_QB):
                p_sb = psbp.tile([128, NT * QB_W], bf16, tag="p_sb")
                pv0 = pvpp.tile([128, 129], f32, tag="pv0")
                pv1 = pvpp.tile([128, 129], f32, tag="pv1")
                qk_t = {}

                def emit_qk(g, qb=qb, qk_t=qk_t):
                    t = qkpp.tile([128, GW], f32, tag="qk")
                    for j4 in range(KCG):
                        j = KCG * g + j4
                        nc.tensor.matmul(
                            out=t[:, j4 * QB_W:(j4 + 1) * QB_W],
                            lhsT=xt[:, j * 128:(j + 1) * 128],
                            rhs=xtq[:, qb * QB_W:(qb + 1) * QB_W],
                            start=True, stop=True)
                    qk_t[g] = t

                emit_qk(0)
                emit_qk(1)
                for g in range(N_G):
                    if g + 2 < N_G:
                        emit_qk(g + 2)
                    pslice = p_sb[:, g * GW:(g + 1) * GW]
                    nc.scalar.activation(out=pslice, in_=qk_t[g][:], func=AF.Exp)
                    del qk_t[g]
                    for j4 in range(KCG):
                        j = KCG * g + j4
                        for qs, pv in ((0, pv0), (1, pv1)):
                            nc.tensor.matmul(
                                out=pv[:],
                                lhsT=p_sb[:, j * QB_W + qs * 128:
                                          j * QB_W + (qs + 1) * 128],
                                rhs=mp[:, j * MPW:j * MPW + 2 * D + 1],
                                start=(j == 0), stop=(j == NT - 1))
                for qs, pv in ((0, pv0), (1, pv1)):
                    o_t = ep2.tile([128, 129], f32, tag=f"o_t{qs}")
                    nc.vector.tensor_copy(out=o_t[:], in_=pv[:])
                    r0 = qb * QB_W + qs * 128
                    nc.sync.dma_start(out=out_d[r0:r0 + 128, :], in_=o_t[:])

    nc.compile()
    return nc


def _emulate_x16(mag, phase):
    """Bit-accurate emulation of the device's f16 feature matrix
    X = [mag*cos(phase) | mag*sin(phase)] (ARW in f32, Sin table ~= np.sin,
    f16 trig and f16 mag multiplied with f32 internals, f16 result)."""
    ph = np.asarray(phase, np.float32)
    out = np.empty((mag.shape[0], 2 * D), np.float16)
    magh = np.asarray(mag, np.float32).astype(np.float16).astype(np.float32)
    for half, shift in ((0, np.float32(math.pi / 2)), (1, np.float32(0.0))):
        y = ph + shift
        y = y + np.float32(2 * math.pi) * ((y < -np.float32(math.pi)).astype(np.float32)
                                           - (y > np.float32(math.pi)).astype(np.float32))
        t16 = np.sin(y.astype(np.float64)).astype(np.float16).astype(np.float32)
        out[:, half * D:(half + 1) * D] = (t16 * magh).astype(np.float16)
    return out


def _prep_corrections(mag, phase, edge_index, edge_attr, W, b):
    """Dedup (last wins) and precompute per-edge host corrections:
    subtract the device's emulated unbiased weight, add the true biased one."""
    src = np.asarray(edge_index[0], dtype=np.int64)
    dst = np.asarray(edge_index[1], dtype=np.int64)
    keys = src * N + dst
    order = np.argsort(keys, kind="stable")
    ks = keys[order]
    run_last = np.flatnonzero(np.r_[ks[1:] != ks[:-1], True])
    kept = order[run_last]  # stable sort => last occurrence per duplicate key
    s, d = src[kept], dst[kept]
    attr = np.asarray(edge_attr, dtype=np.float64)[kept]
    es = attr @ np.asarray(W, np.float64).sum(axis=0) + \
        np.asarray(b, np.float64).sum()

    X16 = _emulate_x16(mag, phase).astype(np.float32)
    S16 = np.einsum("ek,ek->e", X16[s], X16[d], dtype=np.float64)
    S16 = S16.astype(np.float32)
    w_old = np.asarray(np.exp(S16), dtype=ml_dtypes.bfloat16).astype(np.float64)
    w_new = np.exp(S16.astype(np.float64) + es)

    # device value vectors (bf16 mp) for the subtract side; exact for add
    mag16 = np.asarray(mag, np.float32).astype(ml_dtypes.bfloat16).astype(np.float64)
    ph16 = np.asarray(phase, np.float32).astype(ml_dtypes.bfloat16).astype(np.float64)
    V16 = np.concatenate([mag16, ph16], axis=1)
    Vtrue = np.concatenate([np.asarray(mag, np.float64),
                            np.asarray(phase, np.float64)], axis=1)
    num_corr = w_new[:, None] * Vtrue[d] - w_old[:, None] * V16[d]
    z_corr = w_new - w_old
    return s, num_corr, z_corr


def kernel(mag, phase, edge_index, edge_attr, W, b):
    global LAST_RESULTS
    mag = np.ascontiguousarray(np.asarray(mag, dtype=np.float32))
    phase = np.ascontiguousarray(np.asarray(phase, dtype=np.float32))

    s, num_corr, z_corr = _prep_corrections(mag, phase, edge_index,
                                            edge_attr, W, b)

    if "nc" not in _cache:
        _cache["nc"] = _build()
    nc = _cache["nc"]

    in_maps = []
    for c in range(CORES):
        in_maps.append({
            "mag": mag, "phase": phase,
            "magq": np.ascontiguousarray(mag[c * NQ:(c + 1) * NQ]),
            "phaseq": np.ascontiguousarray(phase[c * NQ:(c + 1) * NQ]),
        })
    res = run_bass_kernel_spmd(nc, in_maps, core_ids=list(range(CORES)))
    LAST_RESULTS = res

    num = np.empty((N, 129), dtype=np.float64)
    for c in range(CORES):
        num[c * NQ:(c + 1) * NQ] = res.results[c]["out"].astype(np.float64)

    # host-side sparse bias corrections (segment sums per query row)
    o = np.argsort(s, kind="stable")
    s_s = s[o]
    starts = np.flatnonzero(np.r_[True, s_s[1:] != s_s[:-1]])
    uq = s_s[starts]
    num[uq, 0:128] += np.add.reduceat(num_corr[o], starts, axis=0)
    num[uq, 128] += np.add.reduceat(z_corr[o], starts)

    z = num[:, 128:129]
    new_mag = (num[:, 0:D] / z).astype(np.float32)
    new_phase = (num[:, D:2 * D] / z).astype(np.float32)
    return new_mag, new_phase


# revision 26
# speedup vs baseline: 1.1803x; 1.1803x over previous
"""ComplexPolarAttention Trainium2 kernel (8-core SPMD, row-sharded), v3.

Math (matching the reference):
  c = mag*cos(phase); s = mag*sin(phase)
  scores = c@c.T + s@s.T + bias     (bias: sparse edge scatter, last-dup-wins)
  attn = softmax(scores, axis=1)
  out = (attn@mag, attn@phase)

v3 design:
  - Device computes UNBIASED attention sums only: P = bf16(exp(S)),
    out = [P.T@mag | P.T@phase | P.T@1] unnormalized (129 cols incl Z).
  - The sparse edge bias is applied ON HOST as an exact correction: for each
    (deduped) edge cell, subtract the device's emulated unbiased weight
    bf16(exp(S16)) and add the true biased weight exp(S16 + es), both against
    the same value vectors. The ACT Exp table matches np.exp to ~1e-5 and the
    f16/bf16 roundings are emulated bit-exactly, so the residual is tiny.
  - Keys use a permuted order: node n -> (tile j = n%64, lane p = n//64) so
    mag/phase load as one contiguous 16KB descriptor per partition.
  - QK software-pipelined 2 groups ahead of PV on the PE; exp runs directly
    PSUM->SBUF(bf16) on the scalar engine (the throughput limiter).
"""
import os
import sys

sys.path.insert(0, "/opt/trn_rl_repo")

# The NTFF profile hook module is missing from this image's antenv package;
# bass_utils imports it unconditionally when tracing. Create it if absent so
# BASS_TRACE=1 works (degrades silently if dirs are read-only).
_HOOK_SRC = '''_hook = None

def set_axon_ntff_profile_hook(hook):
    global _hook
    _hook = hook

def get_axon_ntff_profile_hook():
    return _hook
'''
for _d in ("/opt/trn_rl_repo/antenv", "/root/.axon_site/_ro/trn_rl_repo/antenv"):
    try:
        _p = os.path.join(_d, "axon_hooks.py")
        if os.path.isdir(_d) and not os.path.exists(_p):
            with open(_p, "w") as _f:
                _f.write(_HOOK_SRC)
    except OSError:
        pass

import math
import numpy as np
import ml_dtypes

import concourse.bass as bass
import concourse.mybir as mybir
import concourse.tile as tile
from concourse import bacc
from concourse.bass_utils import run_bass_kernel_spmd
from concourse.masks import make_identity

N, D, E, EDGE_DIM = 8192, 64, 262144, 4
CORES = 8
NQ = N // CORES          # 1024 query rows per core
QB_W = 256               # query block width
N_QB = NQ // QB_W        # 4 query blocks per core
NT = 64                  # key tiles (128 permuted nodes each)
KCG = 4                  # key tiles per group
N_G = NT // KCG          # 16 groups
GW = KCG * QB_W          # 1024 = group tile width
MPW = 132                # padded [mag|phase|ones] chunk stride
NCHUNK = 4               # key trig chunks (16 tiles each)
CJT = NT // NCHUNK       # tiles per chunk

f32 = mybir.dt.float32
f16 = mybir.dt.float16
bf16 = mybir.dt.bfloat16
AF = mybir.ActivationFunctionType
ALU = mybir.AluOpType

_cache = {}
LAST_RESULTS = None


def _build():
    nc = bacc.Bacc("TRN2", target_bir_lowering=False, debug=False,
                   num_devices=CORES)
    mag_d = nc.dram_tensor("mag", (N, D), f32, kind="ExternalInput")
    phase_d = nc.dram_tensor("phase", (N, D), f32, kind="ExternalInput")
    magq_d = nc.dram_tensor("magq", (NQ, D), f32, kind="ExternalInput")
    phaseq_d = nc.dram_tensor("phaseq", (NQ, D), f32, kind="ExternalInput")
    out_d = nc.dram_tensor("out", (NQ, 129), f32, kind="ExternalOutput")

    with tile.TileContext(nc) as tc, \
         tc.tile_pool(name="persist", bufs=1) as pers:
        xt = pers.tile([128, N], f16, tag="xt")          # keys, feature-major
        xtq = pers.tile([128, NQ], f16, tag="xtq")       # queries, feature-major
        mp = pers.tile([128, NT * MPW], bf16, tag="mp")  # [mag|phase|1] per tile
        ident = pers.tile([128, 128], f16, tag="ident")
        make_identity(nc, ident[:])

        # ---- stage A ------------------------------------------------------
        with tc.tile_pool(name="a_big", bufs=1) as bigp, \
             tc.tile_pool(name="a_q", bufs=1) as qp, \
             tc.tile_pool(name="a_ws", bufs=2) as wsp, \
             tc.tile_pool(name="a_trig", bufs=2) as trp, \
             tc.tile_pool(name="a_cs", bufs=2) as csp, \
             tc.tile_pool(name="a_ps", bufs=3, space="PSUM") as apsp:
            # queries: [p = i%128, t = i//128, d] (256B descriptors, small)
            magqb = qp.tile([128, 8 * D], f32, tag="magqb")
            phqb = qp.tile([128, 8 * D], f32, tag="phqb")
            magq_r = magq_d[:].rearrange("(t p) d -> p t d", p=128)
            phq_r = phaseq_d[:].rearrange("(t p) d -> p t d", p=128)
            nc.sync.dma_start(out=magqb[:].rearrange("p (t d) -> p t d", d=D),
                              in_=magq_r)
            nc.sync.dma_start(out=phqb[:].rearrange("p (t d) -> p t d", d=D),
                              in_=phq_r)

            # keys: contiguous per-partition load, node n at (p=n//64, j=n%64)
            magb = bigp.tile([128, NT * D], f32, tag="magb")
            phb = bigp.tile([128, NT * D], f32, tag="phb")
            mag_r = mag_d[:].rearrange("(p j) d -> p j d", p=128)
            pha_r = phase_d[:].rearrange("(p j) d -> p j d", p=128)
            magb3 = magb[:].rearrange("p (j d) -> p j d", d=D)
            phb3 = phb[:].rearrange("p (j d) -> p j d", d=D)
            for h in range(NCHUNK):
                j0, j1 = h * CJT, (h + 1) * CJT
                nc.gpsimd.dma_start(out=magb3[:, j0:j1, :],
                                    in_=mag_r[:, j0:j1, :])
                nc.gpsimd.dma_start(out=phb3[:, j0:j1, :],
                                    in_=pha_r[:, j0:j1, :])

            # query trig -> csq -> 8 transposes -> xtq
            magqb3 = magqb[:].rearrange("p (t d) -> p t d", d=D)
            phqb3 = phqb[:].rearrange("p (t d) -> p t d", d=D)
            maghq = qp.tile([128, 8 * D], f16, tag="maghq")
            nc.vector.tensor_copy(out=maghq[:], in_=magqb[:])
            maghq3 = maghq[:].rearrange("p (t d) -> p t d", d=D)
            wsq = qp.tile([128, 8 * 128], f32, tag="wsq")
            wsq3 = wsq[:].rearrange("p (t x) -> p t x", x=128)
            nc.vector.add_range_wrap(out=wsq3[:, :, 0:D], in_=phqb3[:],
                                     shift=math.pi / 2, bound=math.pi,
                                     period=2 * math.pi)
            nc.vector.add_range_wrap(out=wsq3[:, :, D:2 * D], in_=phqb3[:],
                                     shift=0.0, bound=math.pi,
                                     period=2 * math.pi)
            trigq = qp.tile([128, 8 * 128], f16, tag="trigq")
            nc.scalar.activation(out=trigq[:], in_=wsq[:], func=AF.Sin)
            trigq3 = trigq[:].rearrange("p (t x) -> p t x", x=128)
            csq = qp.tile([128, 8 * 128], f16, tag="csq")
            csq3 = csq[:].rearrange("p (t x) -> p t x", x=128)
            nc.vector.tensor_tensor(out=csq3[:, :, 0:D], in0=trigq3[:, :, 0:D],
                                    in1=maghq3[:], op=ALU.mult)
            nc.vector.tensor_tensor(out=csq3[:, :, D:2 * D],
                                    in0=trigq3[:, :, D:2 * D],
                                    in1=maghq3[:], op=ALU.mult)
            for b in range(2):
                tp = apsp.tile([128, 512], f16, tag="tpq")
                for t in range(4):
                    i = b * 4 + t
                    nc.tensor.transpose(out=tp[:, t * 128:(t + 1) * 128],
                                        in_=csq[:, i * 128:(i + 1) * 128],
                                        identity=ident[:])
                nc.vector.tensor_copy(out=xtq[:, b * 512:(b + 1) * 512],
                                      in_=tp[:])

            # key trig in chunks; ARW (f32) -> Sin (f32->f16) -> f16 mults;
            # transposes in batches of 4, PSUM->SBUF copies on gpsimd; mp
            # slices cast on scalar per chunk.
            mp3 = mp[:].rearrange("p (j c) -> p j c", c=MPW)
            for h in range(NCHUNK):
                j0 = h * CJT
                magh = csp.tile([128, CJT * D], f16, tag="magh")
                nc.vector.tensor_copy(out=magh[:], in_=magb[:, j0 * D:(j0 + CJT) * D])
                magh3 = magh[:].rearrange("p (j d) -> p j d", d=D)
                ws = wsp.tile([128, CJT * 128], f32, tag="ws")
                ws3 = ws[:].rearrange("p (j x) -> p j x", x=128)
                nc.vector.add_range_wrap(out=ws3[:, :, 0:D],
                                         in_=phb3[:, j0:j0 + CJT, :],
                                         shift=math.pi / 2, bound=math.pi,
                                         period=2 * math.pi)
                nc.vector.add_range_wrap(out=ws3[:, :, D:2 * D],
                                         in_=phb3[:, j0:j0 + CJT, :],
                                         shift=0.0, bound=math.pi,
                                         period=2 * math.pi)
                trig = trp.tile([128, CJT * 128], f16, tag="trig")
                nc.scalar.activation(out=trig[:], in_=ws[:], func=AF.Sin)
                trig3 = trig[:].rearrange("p (j x) -> p j x", x=128)
                cs = csp.tile([128, CJT * 128], f16, tag="cs")
                cs3 = cs[:].rearrange("p (j x) -> p j x", x=128)
                nc.vector.tensor_tensor(out=cs3[:, :, 0:D],
                                        in0=trig3[:, :, 0:D],
                                        in1=magh3[:], op=ALU.mult)
                nc.vector.tensor_tensor(out=cs3[:, :, D:2 * D],
                                        in0=trig3[:, :, D:2 * D],
                                        in1=magh3[:], op=ALU.mult)
                for b in range(CJT // 4):
                    tp = apsp.tile([128, 512], f16, tag="tpk")
                    for t in range(4):
                        i = b * 4 + t
                        nc.tensor.transpose(out=tp[:, t * 128:(t + 1) * 128],
                                            in_=cs[:, i * 128:(i + 1) * 128],
                                            identity=ident[:])
                    dst = xt[:, (j0 + b * 4) * 128:(j0 + b * 4 + 4) * 128]
                    if b % 2 == 0:
                        nc.scalar.copy(out=dst, in_=tp[:])
                    else:
                        nc.vector.tensor_copy(out=dst, in_=tp[:])
                # mp slices for this chunk ([mag|phase|1] per key tile)
                nc.scalar.copy(out=mp3[:, j0:j0 + CJT, 0:D],
                               in_=magb3[:, j0:j0 + CJT, :])
                nc.scalar.copy(out=mp3[:, j0:j0 + CJT, D:2 * D],
                               in_=phb3[:, j0:j0 + CJT, :])
                nc.vector.memset(mp3[:, j0:j0 + CJT, 2 * D:2 * D + 1], 1.0)

        # ---- stage B: QK -> exp -> PV (QK pipelined 2 groups ahead)
        with tc.tile_pool(name="b_psb", bufs=2) as psbp, \
             tc.tile_pool(name="b_qk", bufs=3, space="PSUM") as qkpp, \
             tc.tile_pool(name="b_pv", bufs=1, space="PSUM") as pvpp, \
             tc.tile_pool(name="b_ep", bufs=2) as ep2:
            for qb in range(N_QB):
                p_sb = psbp.tile([128, NT * QB_W], bf16, tag="p_sb")
                pv0 = pvpp.tile([128, 129], f32, tag="pv0")
                pv1 = pvpp.tile([128, 129], f32, tag="pv1")
                qk_t = {}

                def emit_qk(g, qb=qb, qk_t=qk_t):
                    t = qkpp.tile([128, GW], f32, tag="qk")
                    for j4 in range(KCG):
                        j = KCG * g + j4
                        nc.tensor.matmul(
                            out=t[:, j4 * QB_W:(j4 + 1) * QB_W],
                            lhsT=xt[:, j * 128:(j + 1) * 128],
                            rhs=xtq[:, qb * QB_W:(qb + 1) * QB_W],
                            start=True, stop=True)
                    qk_t[g] = t

                emit_qk(0)
                emit_qk(1)
                for g in range(N_G):
                    if g + 2 < N_G:
                        emit_qk(g + 2)
                    pslice = p_sb[:, g * GW:(g + 1) * GW]
                    nc.scalar.activation(out=pslice, in_=qk_t[g][:], func=AF.Exp)
                    del qk_t[g]
                    for j4 in range(KCG):
                        j = KCG * g + j4
                        for qs, pv in ((0, pv0), (1, pv1)):
                            nc.tensor.matmul(
                                out=pv[:],
                                lhsT=p_sb[:, j * QB_W + qs * 128:
                                          j * QB_W + (qs + 1) * 128],
                                rhs=mp[:, j * MPW:j * MPW + 2 * D + 1],
                                start=(j == 0), stop=(j == NT - 1))
                for qs, pv in ((0, pv0), (1, pv1)):
                    o_t = ep2.tile([128, 129], f32, tag=f"o_t{qs}")
                    nc.vector.tensor_copy(out=o_t[:], in_=pv[:])
                    r0 = qb * QB_W + qs * 128
                    nc.sync.dma_start(out=out_d[r0:r0 + 128, :], in_=o_t[:])

    nc.compile()
    return nc


def _emulate_x16(mag, phase):
    """Bit-accurate emulation of the device's f16 feature matrix
    X = [mag*cos(phase) | mag*sin(phase)] (ARW in f32, Sin table ~= np.sin,
    f16 trig and f16 mag multiplied with f32 internals, f16 result)."""
    ph = np.asarray(phase, np.float32)
    out = np.empty((mag.shape[0], 2 * D), np.float16)
    magh = np.asarray(mag, np.float32).astype(np.float16).astype(np.float32)
    for half, shift in ((0, np.float32(math.pi / 2)), (1, np.float32(0.0))):
        y = ph + shift
        y = y + np.float32(2 * math.pi) * ((y < -np.float32(math.pi)).astype(np.float32)
                                           - (y > np.float32(math.pi)).astype(np.float32))
        t16 = np.sin(y.astype(np.float64)).astype(np.float16).astype(np.float32)
        out[:, half * D:(half + 1) * D] = (t16 * magh).astype(np.float16)
    return out


def _prep_corrections(mag, phase, edge_index, edge_attr, W, b):
    """Dedup (last wins) and precompute per-edge host corrections:
    subtract the device's emulated unbiased weight, add the true biased one."""
    src = np.asarray(edge_index[0], dtype=np.int64)
    dst = np.asarray(edge_index[1], dtype=np.int64)
    keys = src * N + dst
    order = np.argsort(keys, kind="stable")
    ks = keys[order]
    run_last = np.flatnonzero(np.r_[ks[1:] != ks[:-1], True])
    kept = order[run_last]  # stable sort => last occurrence per duplicate key
    s, d = src[kept], dst[kept]
    attr = np.asarray(edge_attr, dtype=np.float64)[kept]
    es = attr @ np.asarray(W, np.float64).sum(axis=0) + \
        np.asarray(b, np.float64).sum()

    X16 = _emulate_x16(mag, phase).astype(np.float32)
    S16 = np.einsum("ek,ek->e", X16[s], X16[d], dtype=np.float64)
    S16 = S16.astype(np.float32)
    w_old = np.asarray(np.exp(S16), dtype=ml_dtypes.bfloat16).astype(np.float64)
    w_new = np.exp(S16.astype(np.float64) + es)

    # device value vectors (bf16 mp) for the subtract side; exact for add
    mag16 = np.asarray(mag, np.float32).astype(ml_dtypes.bfloat16).astype(np.float64)
    ph16 = np.asarray(phase, np.float32).astype(ml_dtypes.bfloat16).astype(np.float64)
    V16 = np.concatenate([mag16, ph16], axis=1)
    Vtrue = np.concatenate([np.asarray(mag, np.float64),
                            np.asarray(phase, np.float64)], axis=1)
    num_corr = w_new[:, None] * Vtrue[d] - w_old[:, None] * V16[d]
    z_corr = w_new - w_old
    return s, num_corr, z_corr


def kernel(mag, phase, edge_index, edge_attr, W, b):
    global LAST_RESULTS
    mag = np.ascontiguousarray(np.asarray(mag, dtype=np.float32))
    phase = np.ascontiguousarray(np.asarray(phase, dtype=np.float32))

    s, num_corr, z_corr = _prep_corrections(mag, phase, edge_index,
                                            edge_attr, W, b)

    if "nc" not in _cache:
        _cache["nc"] = _build()
    nc = _cache["nc"]

    in_maps = []
    for c in range(CORES):
        in_maps.append({
            "mag": mag, "phase": phase,
            "magq": np.ascontiguousarray(mag[c * NQ:(c + 1) * NQ]),
            "phaseq": np.ascontiguousarray(phase[c * NQ:(c + 1) * NQ]),
        })
    res = run_bass_kernel_spmd(nc, in_maps, core_ids=list(range(CORES)))
    LAST_RESULTS = res

    num = np.empty((N, 129), dtype=np.float64)
    for c in range(CORES):
        num[c * NQ:(c + 1) * NQ] = res.results[c]["out"].astype(np.float64)

    # host-side sparse bias corrections (segment sums per query row)
    o = np.argsort(s, kind="stable")
    s_s = s[o]
    starts = np.flatnonzero(np.r_[True, s_s[1:] != s_s[:-1]])
    uq = s_s[starts]
    num[uq, 0:128] += np.add.reduceat(num_corr[o], starts, axis=0)
    num[uq, 128] += np.add.reduceat(z_corr[o], starts)

    z = num[:, 128:129]
    new_mag = (num[:, 0:D] / z).astype(np.float32)
    new_phase = (num[:, D:2 * D] / z).astype(np.float32)
    return new_mag, new_phase
